# revision 13
# baseline (speedup 1.0000x reference)
"""Block-sparse linear (DSD) y = x @ W^T on 8 Trainium2 NeuronCores.

Math: W is [4096, 4096] built from 4096 nonzero 32x32 blocks at block
coords (ri, ci) on a 128x128 block grid. The reference layout is
(gi + gj) % 4 == 0, so block-rows with equal (gi mod 4) share an identical
set of 32 block-columns: the sparse matmul decomposes into 4 dense
[tokens x 1024] @ [1024 x 1024] matmuls -- exactly the 25%-density FLOPs.

Sharding (8 cores): hybrid -- 4 residue groups x 2 token halves. Core
c = g*2 + h computes y[h-half, outcols(g)] = x[h-half, incols(g)] @ Wg^T.
Each core reads only its group's weights (4MB) + its token half of the
gathered x (16MB) and writes 16MB -- 36MB of HBM traffic under ~109us of
f32r PE work. No collectives.

Numerics: matmul operands go through the PE's float32r path (fp32 bits,
reduced-precision multiply, fp32 accumulate in PSUM). Measured on HW:
rel L2 error ~1.5e-4 for K=1024 accumulation (vs ~2.3e-3 for bf16).

Host work: pack w_blocks into dense [in, out] panels, transpose/gather x,
un-permute output columns. If ri/ci do not match the 4-group structure,
fall back to a dense-W fp32 kernel (correct for any layout).
"""

import sys

import numpy as np

if "/opt/trn_rl_repo" not in sys.path:
    sys.path.insert(0, "/opt/trn_rl_repo")

import concourse.bacc as bacc
import concourse.mybir as mybir
from concourse.bass_utils import run_bass_kernel_spmd
from concourse.tile import TileContext

BLOCK = 32
HO = 128  # out_features // BLOCK
WO = 128  # in_features  // BLOCK
N_TOK = 8192
N_CORES = 8
TOKH = N_TOK // 2  # tokens per core (token half)

# matmul operand dtype: "f32r" = fast fp32 path (1 cycle/row, ~1.5e-4 rel
# err), "f32" = exact fp32 (4 cycles/row)
MM_DTYPE = "f32r"

# set by test.py to capture a profile; harness never touches these
_TRACE = False
LAST_RESULT = None


def _mm_dt():
    return {
        "f32": mybir.dt.float32,
        "f32r": mybir.dt.float32r,
    }[MM_DTYPE]


def _build_hybrid(dt_in):
    """One residue group per core: y[TOKH, 1024] = xg[TOKH, 1024] @ Wg^T.

    Inputs:  xT [8, 128, TOKH]  (gathered x^T: contraction on partitions)
             wT [8, 128, 1024]  (Wg^T panels: [in, out])
    Output:  y  [TOKH, 1024] fp32, out columns in group-local order.
    """
    nc = bacc.Bacc()
    f32 = mybir.dt.float32
    KT, Mg = 8, 1024
    MCH = 4  # m-tiles (128 tokens) per x chunk
    NCH = TOKH // (MCH * 128)  # chunks
    xT = nc.dram_tensor("xT", [KT, 128, TOKH], dt_in, kind="ExternalInput")
    wT = nc.dram_tensor("wT", [KT, 128, Mg], dt_in, kind="ExternalInput")
    y = nc.dram_tensor("y", [TOKH, Mg], f32, kind="ExternalOutput")

    def store(ps, row, n):
        # copy on DVE, store-issue on ScalarE: keeps the Sync engine's
        # instruction stream free for x-chunk load issues
        ob = op.tile([128, 512], f32, tag="ob")
        nc.vector.tensor_copy(ob[:], ps[:])
        nc.scalar.dma_start(
            out=y[row : row + 128, n * 512 : (n + 1) * 512], in_=ob[:]
        )

    with TileContext(nc) as tc:
        with (
            tc.tile_pool(name="wp", bufs=KT) as wp,
            tc.tile_pool(name="xp", bufs=5 * KT) as xp,
            tc.tile_pool(name="pp", bufs=8, space="PSUM") as pp,
            tc.tile_pool(name="op", bufs=8) as op,
        ):
            # PE warm-up: the HAM clock gate holds the PE at 1.2 GHz until
            # it has been busy ~3.4us. Spin junk matmuls on a memset tile
            # while the first DMAs stream so the real stream starts at
            # 2.4 GHz.
            wu = xp.tile([128, 128], f32, tag="warm", bufs=1)
            nc.gpsimd.memset(wu[:], 0.0)
            wups = pp.tile([128, 128], f32, tag="ps")
            NWU = 6  # each fp32 matmul lowers to a LO/HI pair on HW
            for i in range(NWU):
                nc.tensor.matmul(
                    wups[:],
                    lhsT=wu[:],
                    rhs=wu[:],
                    start=(i == 0),
                    stop=(i == NWU - 1),
                )

            # interleave w / x-chunk0 loads k-by-k so the PE can start on
            # k-row 0 after ~0.75MB instead of waiting for all 6MB
            wt, xt0 = [], []
            for k in range(KT):
                wk = wp.tile([128, Mg], dt_in, tag="w")
                nc.sync.dma_start(out=wk[:], in_=wT[k])
                wt.append(wk)
                xk = xp.tile([128, MCH * 128], dt_in, tag="x")
                nc.sync.dma_start(out=xk[:], in_=xT[k, :, 0 : MCH * 128])
                xt0.append(xk)

            def emit_chunk(ch, xt, k_outer):
                if k_outer:
                    # k-outer over all 8 psum banks: each k-row is unlocked
                    # by (w[k], x[k]) alone, so compute streams with DMA
                    # while the chunk's tiles are still arriving
                    psc = [
                        [
                            pp.tile(
                                [128, 512], f32, tag="ps", name=f"ps{ch}_{m}_{n}"
                            )
                            for n in range(2)
                        ]
                        for m in range(MCH)
                    ]
                    for k in range(KT):
                        for m in range(MCH):
                            for n in range(2):
                                nc.tensor.matmul(
                                    psc[m][n][:],
                                    lhsT=xt[k][:, m * 128 : (m + 1) * 128],
                                    rhs=wt[k][:, n * 512 : (n + 1) * 512],
                                    start=(k == 0),
                                    stop=(k == KT - 1),
                                )
                    for m in range(MCH):
                        for n in range(2):
                            store(psc[m][n], (ch * MCH + m) * 128, n)
                else:
                    # k-inner (psum-sequential): staggers PSUM-bank reuse so
                    # copies overlap the next psum's matmuls
                    for m in range(MCH):
                        row = (ch * MCH + m) * 128
                        for n in range(2):
                            ps = pp.tile([128, 512], f32, tag="ps")
                            for k in range(KT):
                                nc.tensor.matmul(
                                    ps[:],
                                    lhsT=xt[k][:, m * 128 : (m + 1) * 128],
                                    rhs=wt[k][:, n * 512 : (n + 1) * 512],
                                    start=(k == 0),
                                    stop=(k == KT - 1),
                                )
                            store(ps, row, n)

            emit_chunk(0, xt0, k_outer=True)
            for ch in range(1, NCH):
                xt = []
                for k in range(KT):
                    xk = xp.tile([128, MCH * 128], dt_in, tag="x")
                    nc.sync.dma_start(
                        out=xk[:],
                        in_=xT[k, :, ch * MCH * 128 : (ch + 1) * MCH * 128],
                    )
                    xt.append(xk)
                emit_chunk(ch, xt, k_outer=(ch == 1))
    nc.compile()
    return nc


def _build_dense():
    """Fallback: y = x @ W^T with dense W [4096, 4096] in fp32; any layout.

    Inputs:  xT [32, 128, 1024]   (x transposed, 1024 tokens/core)
             wT [32, 128, 4096]   (W^T = [in, out])
    Output:  y  [1024, 4096]
    """
    nc = bacc.Bacc()
    f32 = mybir.dt.float32
    KT, NO, TOK = 32, 4096, N_TOK // N_CORES
    xT = nc.dram_tensor("xT", [KT, 128, TOK], f32, kind="ExternalInput")
    wT = nc.dram_tensor("wT", [KT, 128, NO], f32, kind="ExternalInput")
    y = nc.dram_tensor("y", [TOK, NO], f32, kind="ExternalOutput")
    MT = TOK // 128
    NT = NO // 512

    with TileContext(nc) as tc:
        with (
            tc.tile_pool(name="xp", bufs=KT) as xp,
            tc.tile_pool(name="wp", bufs=2 * KT) as wp,
            tc.tile_pool(name="pp", bufs=8, space="PSUM") as pp,
            tc.tile_pool(name="op", bufs=8) as op,
        ):
            xt = []
            for k in range(KT):
                xk = xp.tile([128, TOK], f32, tag="x")
                nc.sync.dma_start(out=xk[:], in_=xT[k])
                xt.append(xk)
            # stream W one 512-wide out-panel at a time
            for n in range(NT):
                wt = []
                for k in range(KT):
                    wk = wp.tile([128, 512], f32, tag="w")
                    nc.sync.dma_start(out=wk[:], in_=wT[k, :, n * 512 : (n + 1) * 512])
                    wt.append(wk)
                for m in range(MT):
                    ps = pp.tile([128, 512], f32, tag="ps")
                    for k in range(KT):
                        nc.tensor.matmul(
                            ps[:],
                            lhsT=xt[k][:, m * 128 : (m + 1) * 128],
                            rhs=wt[k][:],
                            start=(k == 0),
                            stop=(k == KT - 1),
                        )
                    ob = op.tile([128, 512], f32, tag="ob")
                    nc.vector.tensor_copy(ob[:], ps[:])
                    nc.sync.dma_start(
                        out=y[m * 128 : (m + 1) * 128, n * 512 : (n + 1) * 512],
                        in_=ob[:],
                    )
    nc.compile()
    return nc


def _detect_groups(ri, ci):
    """Group block-rows that share an identical block-column set.

    Returns (groups, blk_id) with exactly 4 groups of 32 rows x 32 cols,
    or None if the structure doesn't decompose that way.
    """
    ri = np.asarray(ri)
    ci = np.asarray(ci)
    if len(ri) != HO * WO // 4:
        return None
    pairs = set(zip(ri.tolist(), ci.tolist()))
    if len(pairs) != len(ri):
        return None  # duplicate blocks: last-write-wins semantics -> fallback
    blk_id = np.full((HO, WO), -1, dtype=np.int64)
    blk_id[ri, ci] = np.arange(len(ri))
    col_sets = {}
    for g in range(HO):
        cols = np.sort(ci[ri == g])
        col_sets.setdefault(tuple(cols.tolist()), []).append(g)
    groups = []
    for cols, rows in col_sets.items():
        if len(rows) != 32 or len(cols) != 32:
            return None
        groups.append((np.array(rows), np.array(cols)))
    if len(groups) != 4:
        return None
    return groups, blk_id


def kernel(x, w_blocks, ri, ci):
    global LAST_RESULT
    x = np.asarray(x, dtype=np.float32)
    w_blocks = np.asarray(w_blocks, dtype=np.float32)
    ri = np.asarray(ri, dtype=np.int64)
    ci = np.asarray(ci, dtype=np.int64)

    det = _detect_groups(ri, ci)
    core_ids = list(range(N_CORES))

    if det is not None:
        groups, blk_id = det
        KT, Mg = 8, 1024
        Kg = KT * 128
        wT = np.empty((4, KT, 128, Mg), dtype=np.float32)
        perm_out = np.empty((4, Mg), dtype=np.int64)
        perm_in = np.empty((4, Kg), dtype=np.int64)
        for g, (rows, cols) in enumerate(groups):
            idx = blk_id[np.ix_(rows, cols)]  # [32, 32] block ids
            # Wg[p, q, bi, bj] = W[rows[p]*32+bi, cols[q]*32+bj]
            # -> [q*32+bj, p*32+bi] = Wg^T as [in, out]
            wT[g] = w_blocks[idx].transpose(1, 3, 0, 2).reshape(KT, 128, Mg)
            perm_out[g] = (rows[:, None] * BLOCK + np.arange(BLOCK)).ravel()
            perm_in[g] = (cols[:, None] * BLOCK + np.arange(BLOCK)).ravel()
        xTfull = np.ascontiguousarray(x.T)  # [in, tok]
        xg = xTfull[perm_in.reshape(-1)].reshape(4, Kg, N_TOK)
        in_maps = []
        for c in core_ids:
            g, h = c // 2, c % 2
            xc = np.ascontiguousarray(
                xg[g, :, h * TOKH : (h + 1) * TOKH]
            ).reshape(KT, 128, TOKH)
            in_maps.append({"xT": xc, "wT": wT[g]})
        nc = _build_hybrid(_mm_dt())
        res = run_bass_kernel_spmd(nc, in_maps, core_ids, trace=_TRACE)
        LAST_RESULT = res
        y = np.empty((N_TOK, HO * BLOCK), dtype=np.float32)
        for c in core_ids:
            g, h = c // 2, c % 2
            y[h * TOKH : (h + 1) * TOKH][:, perm_out[g]] = res.results[c]["y"]
        return y

    # ---- dense fallback: scatter blocks into dense W (last write wins)
    TOK = N_TOK // N_CORES
    Wb = np.zeros((HO, WO, BLOCK, BLOCK), dtype=np.float32)
    Wb[ri, ci] = w_blocks
    W = Wb.transpose(0, 2, 1, 3).reshape(HO * BLOCK, WO * BLOCK)
    wT = np.ascontiguousarray(W.T).reshape(32, 128, 4096)
    xTfull = np.ascontiguousarray(x.T)
    in_maps = []
    for c in core_ids:
        xc = np.ascontiguousarray(xTfull[:, c * TOK : (c + 1) * TOK]).reshape(
            32, 128, TOK
        )
        in_maps.append({"xT": xc, "wT": wT})
    nc = _build_dense()
    res = run_bass_kernel_spmd(nc, in_maps, core_ids, trace=_TRACE)
    LAST_RESULT = res
    return np.concatenate([r["y"] for r in res.results], axis=0)


# revision 16
# speedup vs baseline: 1.0102x; 1.0102x over previous
"""Block-sparse linear (DSD) y = x @ W^T on 8 Trainium2 NeuronCores.

Math: W is [4096, 4096] built from 4096 nonzero 32x32 blocks at block
coords (ri, ci) on a 128x128 block grid. The reference layout is
(gi + gj) % 4 == 0, so block-rows with equal (gi mod 4) share an identical
set of 32 block-columns: the sparse matmul decomposes into 4 dense
[tokens x 1024] @ [1024 x 1024] matmuls -- exactly the 25%-density FLOPs.

Sharding (8 cores): hybrid -- 4 residue groups x 2 token halves. Core
c = g*2 + h computes y[h-half, outcols(g)] = x[h-half, incols(g)] @ Wg^T.
Each core reads only its group's weights (4MB) + its token half of the
gathered x (16MB) and writes 16MB -- 36MB of HBM traffic under ~109us of
f32r PE work. No collectives.

Numerics: matmul operands go through the PE's float32r path (fp32 bits,
reduced-precision multiply, fp32 accumulate in PSUM). Measured on HW:
rel L2 error ~1.5e-4 for K=1024 accumulation (vs ~2.3e-3 for bf16).

Host work: pack w_blocks into dense [in, out] panels, transpose/gather x,
un-permute output columns. If ri/ci do not match the 4-group structure,
fall back to a dense-W fp32 kernel (correct for any layout).
"""

import sys

import numpy as np

if "/opt/trn_rl_repo" not in sys.path:
    sys.path.insert(0, "/opt/trn_rl_repo")

import concourse.bacc as bacc
import concourse.mybir as mybir
from concourse.bass_utils import run_bass_kernel_spmd
from concourse.tile import TileContext

# bass_utils imports antenv.axon_hooks when tracing is requested (e.g. via
# BASS_TRACE=1). Some images lack that module; provide an inert stub so a
# trace request degrades to "no trace" instead of crashing.
try:
    import antenv.axon_hooks  # noqa: F401
except Exception:  # pragma: no cover
    import types

    try:
        import antenv

        _hooks = types.ModuleType("antenv.axon_hooks")
        _hooks._h = None
        _hooks.set_axon_ntff_profile_hook = lambda h: setattr(_hooks, "_h", h)
        _hooks.get_axon_ntff_profile_hook = lambda: _hooks._h
        sys.modules["antenv.axon_hooks"] = _hooks
        antenv.axon_hooks = _hooks
    except Exception:
        pass

BLOCK = 32
HO = 128  # out_features // BLOCK
WO = 128  # in_features  // BLOCK
N_TOK = 8192
N_CORES = 8
TOKH = N_TOK // 2  # tokens per core (token half)

# matmul operand dtype: "f32r" = fast fp32 path (1 cycle/row, ~1.5e-4 rel
# err), "f32" = exact fp32 (4 cycles/row)
MM_DTYPE = "f32r"

# set by test.py to capture a profile; harness never touches these
_TRACE = False
LAST_RESULT = None


def _mm_dt():
    return {
        "f32": mybir.dt.float32,
        "f32r": mybir.dt.float32r,
    }[MM_DTYPE]


def _build_hybrid(dt_in):
    """One residue group per core: y[TOKH, 1024] = xg[TOKH, 1024] @ Wg^T.

    Inputs:  xT [8, 128, TOKH]  (gathered x^T: contraction on partitions)
             wT [8, 128, 1024]  (Wg^T panels: [in, out])
    Output:  y  [TOKH, 1024] fp32, out columns in group-local order.
    """
    nc = bacc.Bacc()
    f32 = mybir.dt.float32
    KT, Mg = 8, 1024
    MCH = 4  # m-tiles (128 tokens) per x chunk
    NCH = TOKH // (MCH * 128)  # chunks
    xT = nc.dram_tensor("xT", [KT, 128, TOKH], dt_in, kind="ExternalInput")
    wT = nc.dram_tensor("wT", [KT, 128, Mg], dt_in, kind="ExternalInput")
    y = nc.dram_tensor("y", [TOKH, Mg], f32, kind="ExternalOutput")

    def store(ps, row, n):
        # copy on DVE, store-issue on ScalarE: keeps the Sync engine's
        # instruction stream free for x-chunk load issues
        ob = op.tile([128, 512], f32, tag="ob")
        nc.vector.tensor_copy(ob[:], ps[:])
        nc.scalar.dma_start(
            out=y[row : row + 128, n * 512 : (n + 1) * 512], in_=ob[:]
        )

    with TileContext(nc) as tc:
        with (
            tc.tile_pool(name="wp", bufs=KT) as wp,
            tc.tile_pool(name="xp", bufs=5 * KT) as xp,
            tc.tile_pool(name="pp", bufs=8, space="PSUM") as pp,
            tc.tile_pool(name="op", bufs=8) as op,
        ):
            # PE warm-up: the HAM clock gate holds the PE at 1.2 GHz until
            # it has been busy ~3.4us. Spin junk matmuls on a memset tile
            # while the first DMAs stream so the real stream starts at
            # 2.4 GHz.
            wu = xp.tile([128, 128], f32, tag="warm", bufs=1)
            nc.gpsimd.memset(wu[:], 0.0)
            wups = pp.tile([128, 128], f32, tag="ps")
            NWU = 4  # each fp32 matmul lowers to a LO/HI pair on HW
            for i in range(NWU):
                nc.tensor.matmul(
                    wups[:],
                    lhsT=wu[:],
                    rhs=wu[:],
                    start=(i == 0),
                    stop=(i == NWU - 1),
                )

            # interleave w / x-chunk0 loads k-by-k so the PE can start on
            # k-row 0 after ~0.75MB instead of waiting for all 6MB
            wt, xt0 = [], []
            for k in range(KT):
                wk = wp.tile([128, Mg], dt_in, tag="w")
                nc.sync.dma_start(out=wk[:], in_=wT[k])
                wt.append(wk)
                xk = xp.tile([128, MCH * 128], dt_in, tag="x")
                nc.sync.dma_start(out=xk[:], in_=xT[k, :, 0 : MCH * 128])
                xt0.append(xk)

            def emit_chunk(ch, xt, k_outer):
                if k_outer:
                    # k-outer over all 8 psum banks: each k-row is unlocked
                    # by (w[k], x[k]) alone, so compute streams with DMA
                    # while the chunk's tiles are still arriving
                    psc = [
                        [
                            pp.tile(
                                [128, 512], f32, tag="ps", name=f"ps{ch}_{m}_{n}"
                            )
                            for n in range(2)
                        ]
                        for m in range(MCH)
                    ]
                    for k in range(KT):
                        for m in range(MCH):
                            for n in range(2):
                                nc.tensor.matmul(
                                    psc[m][n][:],
                                    lhsT=xt[k][:, m * 128 : (m + 1) * 128],
                                    rhs=wt[k][:, n * 512 : (n + 1) * 512],
                                    start=(k == 0),
                                    stop=(k == KT - 1),
                                )
                    for m in range(MCH):
                        for n in range(2):
                            store(psc[m][n], (ch * MCH + m) * 128, n)
                else:
                    # k-inner (psum-sequential): staggers PSUM-bank reuse so
                    # copies overlap the next psum's matmuls
                    for m in range(MCH):
                        row = (ch * MCH + m) * 128
                        for n in range(2):
                            ps = pp.tile([128, 512], f32, tag="ps")
                            for k in range(KT):
                                nc.tensor.matmul(
                                    ps[:],
                                    lhsT=xt[k][:, m * 128 : (m + 1) * 128],
                                    rhs=wt[k][:, n * 512 : (n + 1) * 512],
                                    start=(k == 0),
                                    stop=(k == KT - 1),
                                )
                            store(ps, row, n)

            emit_chunk(0, xt0, k_outer=True)
            for ch in range(1, NCH):
                xt = []
                for k in range(KT):
                    xk = xp.tile([128, MCH * 128], dt_in, tag="x")
                    nc.sync.dma_start(
                        out=xk[:],
                        in_=xT[k, :, ch * MCH * 128 : (ch + 1) * MCH * 128],
                    )
                    xt.append(xk)
                emit_chunk(ch, xt, k_outer=(ch == 1))
    nc.compile()
    return nc


def _build_dense():
    """Fallback: y = x @ W^T with dense W [4096, 4096] in fp32; any layout.

    Inputs:  xT [32, 128, 1024]   (x transposed, 1024 tokens/core)
             wT [32, 128, 4096]   (W^T = [in, out])
    Output:  y  [1024, 4096]
    """
    nc = bacc.Bacc()
    f32 = mybir.dt.float32
    KT, NO, TOK = 32, 4096, N_TOK // N_CORES
    xT = nc.dram_tensor("xT", [KT, 128, TOK], f32, kind="ExternalInput")
    wT = nc.dram_tensor("wT", [KT, 128, NO], f32, kind="ExternalInput")
    y = nc.dram_tensor("y", [TOK, NO], f32, kind="ExternalOutput")
    MT = TOK // 128
    NT = NO // 512

    with TileContext(nc) as tc:
        with (
            tc.tile_pool(name="xp", bufs=2 * KT) as xp,
            tc.tile_pool(name="wp", bufs=KT) as wp,
            tc.tile_pool(name="pp", bufs=8, space="PSUM") as pp,
            tc.tile_pool(name="op", bufs=8) as op,
        ):
            # n-outer: one 512-wide W panel (32 k-tiles = 64KB/partition)
            # resident at a time; x streamed per m-tile (re-read per panel)
            for n in range(NT):
                wt = []
                for k in range(KT):
                    wk = wp.tile([128, 512], f32, tag="w")
                    nc.sync.dma_start(out=wk[:], in_=wT[k, :, n * 512 : (n + 1) * 512])
                    wt.append(wk)
                for m in range(MT):
                    xt = []
                    for k in range(KT):
                        xk = xp.tile([128, 128], f32, tag="x")
                        nc.sync.dma_start(
                            out=xk[:], in_=xT[k, :, m * 128 : (m + 1) * 128]
                        )
                        xt.append(xk)
                    ps = pp.tile([128, 512], f32, tag="ps")
                    for k in range(KT):
                        nc.tensor.matmul(
                            ps[:],
                            lhsT=xt[k][:],
                            rhs=wt[k][:],
                            start=(k == 0),
                            stop=(k == KT - 1),
                        )
                    ob = op.tile([128, 512], f32, tag="ob")
                    nc.vector.tensor_copy(ob[:], ps[:])
                    nc.scalar.dma_start(
                        out=y[m * 128 : (m + 1) * 128, n * 512 : (n + 1) * 512],
                        in_=ob[:],
                    )
    nc.compile()
    return nc


def _detect_groups(ri, ci):
    """Group block-rows that share an identical block-column set.

    Returns (groups, blk_id) with exactly 4 groups of 32 rows x 32 cols,
    or None if the structure doesn't decompose that way.
    """
    ri = np.asarray(ri)
    ci = np.asarray(ci)
    if len(ri) != HO * WO // 4:
        return None
    pairs = set(zip(ri.tolist(), ci.tolist()))
    if len(pairs) != len(ri):
        return None  # duplicate blocks: last-write-wins semantics -> fallback
    blk_id = np.full((HO, WO), -1, dtype=np.int64)
    blk_id[ri, ci] = np.arange(len(ri))
    col_sets = {}
    for g in range(HO):
        cols = np.sort(ci[ri == g])
        col_sets.setdefault(tuple(cols.tolist()), []).append(g)
    groups = []
    for cols, rows in col_sets.items():
        if len(rows) != 32 or len(cols) != 32:
            return None
        groups.append((np.array(rows), np.array(cols)))
    if len(groups) != 4:
        return None
    return groups, blk_id


def kernel(x, w_blocks, ri, ci):
    global LAST_RESULT
    x = np.asarray(x, dtype=np.float32)
    w_blocks = np.asarray(w_blocks, dtype=np.float32)
    ri = np.asarray(ri, dtype=np.int64)
    ci = np.asarray(ci, dtype=np.int64)

    det = _detect_groups(ri, ci)
    core_ids = list(range(N_CORES))

    if det is not None:
        groups, blk_id = det
        KT, Mg = 8, 1024
        Kg = KT * 128
        wT = np.empty((4, KT, 128, Mg), dtype=np.float32)
        perm_out = np.empty((4, Mg), dtype=np.int64)
        perm_in = np.empty((4, Kg), dtype=np.int64)
        for g, (rows, cols) in enumerate(groups):
            idx = blk_id[np.ix_(rows, cols)]  # [32, 32] block ids
            # Wg[p, q, bi, bj] = W[rows[p]*32+bi, cols[q]*32+bj]
            # -> [q*32+bj, p*32+bi] = Wg^T as [in, out]
            wT[g] = w_blocks[idx].transpose(1, 3, 0, 2).reshape(KT, 128, Mg)
            perm_out[g] = (rows[:, None] * BLOCK + np.arange(BLOCK)).ravel()
            perm_in[g] = (cols[:, None] * BLOCK + np.arange(BLOCK)).ravel()
        xTfull = np.ascontiguousarray(x.T)  # [in, tok]
        xg = xTfull[perm_in.reshape(-1)].reshape(4, Kg, N_TOK)
        in_maps = []
        for c in core_ids:
            g, h = c // 2, c % 2
            xc = np.ascontiguousarray(
                xg[g, :, h * TOKH : (h + 1) * TOKH]
            ).reshape(KT, 128, TOKH)
            in_maps.append({"xT": xc, "wT": wT[g]})
        nc = _build_hybrid(_mm_dt())
        res = run_bass_kernel_spmd(nc, in_maps, core_ids, trace=_TRACE)
        LAST_RESULT = res
        y = np.empty((N_TOK, HO * BLOCK), dtype=np.float32)
        for c in core_ids:
            g, h = c // 2, c % 2
            y[h * TOKH : (h + 1) * TOKH][:, perm_out[g]] = res.results[c]["y"]
        return y

    # ---- dense fallback: scatter blocks into dense W (last write wins)
    TOK = N_TOK // N_CORES
    Wb = np.zeros((HO, WO, BLOCK, BLOCK), dtype=np.float32)
    Wb[ri, ci] = w_blocks
    W = Wb.transpose(0, 2, 1, 3).reshape(HO * BLOCK, WO * BLOCK)
    wT = np.ascontiguousarray(W.T).reshape(32, 128, 4096)
    xTfull = np.ascontiguousarray(x.T)
    in_maps = []
    for c in core_ids:
        xc = np.ascontiguousarray(xTfull[:, c * TOK : (c + 1) * TOK]).reshape(
            32, 128, TOK
        )
        in_maps.append({"xT": xc, "wT": wT})
    nc = _build_dense()
    res = run_bass_kernel_spmd(nc, in_maps, core_ids, trace=_TRACE)
    LAST_RESULT = res
    return np.concatenate([r["y"] for r in res.results], axis=0)


# revision 18
# speedup vs baseline: 1.0113x; 1.0010x over previous
"""Block-sparse linear (DSD) y = x @ W^T on 8 Trainium2 NeuronCores.

Math: W is [4096, 4096] built from 4096 nonzero 32x32 blocks at block
coords (ri, ci) on a 128x128 block grid. The reference layout is
(gi + gj) % 4 == 0, so block-rows with equal (gi mod 4) share an identical
set of 32 block-columns: the sparse matmul decomposes into 4 dense
[tokens x 1024] @ [1024 x 1024] matmuls -- exactly the 25%-density FLOPs.

Sharding (8 cores): hybrid -- 4 residue groups x 2 token halves. Core
c = g*2 + h computes y[h-half, outcols(g)] = x[h-half, incols(g)] @ Wg^T.
Each core reads only its group's weights (4MB) + its token half of the
gathered x (16MB) and writes 16MB -- 36MB of HBM traffic under ~109us of
f32r PE work. No collectives.

Numerics: matmul operands go through the PE's float32r path (fp32 bits,
reduced-precision multiply, fp32 accumulate in PSUM). Measured on HW:
rel L2 error ~1.5e-4 for K=1024 accumulation (vs ~2.3e-3 for bf16).

Host work: pack w_blocks into dense [in, out] panels, transpose/gather x,
un-permute output columns. If ri/ci do not match the 4-group structure,
fall back to a dense-W fp32 kernel (correct for any layout).
"""

import sys

import numpy as np

if "/opt/trn_rl_repo" not in sys.path:
    sys.path.insert(0, "/opt/trn_rl_repo")

import concourse.bacc as bacc
import concourse.mybir as mybir
from concourse.bass_utils import run_bass_kernel_spmd
from concourse.tile import TileContext

# bass_utils imports antenv.axon_hooks when tracing is requested (e.g. via
# BASS_TRACE=1). Some images lack that module; provide an inert stub so a
# trace request degrades to "no trace" instead of crashing.
try:
    import antenv.axon_hooks  # noqa: F401
except Exception:  # pragma: no cover
    import types

    try:
        import antenv

        _hooks = types.ModuleType("antenv.axon_hooks")
        _hooks._h = None
        _hooks.set_axon_ntff_profile_hook = lambda h: setattr(_hooks, "_h", h)
        _hooks.get_axon_ntff_profile_hook = lambda: _hooks._h
        sys.modules["antenv.axon_hooks"] = _hooks
        antenv.axon_hooks = _hooks
    except Exception:
        pass

BLOCK = 32
HO = 128  # out_features // BLOCK
WO = 128  # in_features  // BLOCK
N_TOK = 8192
N_CORES = 8
TOKH = N_TOK // 2  # tokens per core (token half)

# matmul operand dtype: "f32r" = fast fp32 path (1 cycle/row, ~1.5e-4 rel
# err), "f32" = exact fp32 (4 cycles/row)
MM_DTYPE = "f32r"

# set by test.py to capture a profile; harness never touches these
_TRACE = False
LAST_RESULT = None


def _mm_dt():
    return {
        "f32": mybir.dt.float32,
        "f32r": mybir.dt.float32r,
    }[MM_DTYPE]


def _build_hybrid(dt_in):
    """One residue group per core: y[TOKH, 1024] = xg[TOKH, 1024] @ Wg^T.

    Inputs:  xT [8, 128, TOKH]  (gathered x^T: contraction on partitions)
             wT [8, 128, 1024]  (Wg^T panels: [in, out])
    Output:  y  [TOKH, 1024] fp32, out columns in group-local order.
    """
    nc = bacc.Bacc()
    f32 = mybir.dt.float32
    KT, Mg = 8, 1024
    MCH = 4  # m-tiles (128 tokens) per x chunk
    NCH = TOKH // (MCH * 128)  # chunks
    xT = nc.dram_tensor("xT", [KT, 128, TOKH], dt_in, kind="ExternalInput")
    wT = nc.dram_tensor("wT", [KT, 128, Mg], dt_in, kind="ExternalInput")
    y = nc.dram_tensor("y", [TOKH, Mg], f32, kind="ExternalOutput")

    def store(ps, row, n):
        # copy on DVE, store-issue on ScalarE: keeps the Sync engine's
        # instruction stream free for x-chunk load issues
        ob = op.tile([128, 512], f32, tag="ob")
        nc.vector.tensor_copy(ob[:], ps[:])
        nc.scalar.dma_start(
            out=y[row : row + 128, n * 512 : (n + 1) * 512], in_=ob[:]
        )

    with TileContext(nc) as tc:
        with (
            tc.tile_pool(name="wp", bufs=2 * KT) as wp,
            tc.tile_pool(name="xp", bufs=5 * KT) as xp,
            tc.tile_pool(name="pp", bufs=8, space="PSUM") as pp,
            tc.tile_pool(name="op", bufs=8) as op,
        ):
            # PE warm-up: the HAM clock gate holds the PE at 1.2 GHz until
            # it has been busy ~3.4us. Spin junk matmuls on a memset tile
            # while the first DMAs stream so the real stream starts at
            # 2.4 GHz.
            wu = xp.tile([128, 128], f32, tag="warm", bufs=1)
            nc.gpsimd.memset(wu[:], 0.0)
            wups = pp.tile([128, 128], f32, tag="ps")
            NWU = 4  # each fp32 matmul lowers to a LO/HI pair on HW
            for i in range(NWU):
                nc.tensor.matmul(
                    wups[:],
                    lhsT=wu[:],
                    rhs=wu[:],
                    start=(i == 0),
                    stop=(i == NWU - 1),
                )

            # interleave w / x-chunk0 loads k-by-k, with w split into
            # 512-wide n-halves, so the first matmul is gated on only
            # ~0.5MB (w[0] n0-half + x[0] chunk) instead of 6MB
            wt, xt0 = [], []
            for k in range(KT):
                wkn = []
                wk0 = wp.tile([128, 512], dt_in, tag="w", name=f"w{k}_0")
                nc.sync.dma_start(out=wk0[:], in_=wT[k, :, 0:512])
                wkn.append(wk0)
                xk = xp.tile([128, MCH * 128], dt_in, tag="x")
                nc.sync.dma_start(out=xk[:], in_=xT[k, :, 0 : MCH * 128])
                xt0.append(xk)
                wk1 = wp.tile([128, 512], dt_in, tag="w", name=f"w{k}_1")
                nc.sync.dma_start(out=wk1[:], in_=wT[k, :, 512:1024])
                wkn.append(wk1)
                wt.append(wkn)

            def emit_chunk(ch, xt, k_outer):
                if k_outer:
                    # k-outer over all 8 psum banks: each k-row is unlocked
                    # by (w[k], x[k]) alone, so compute streams with DMA
                    # while the chunk's tiles are still arriving
                    psc = [
                        [
                            pp.tile(
                                [128, 512], f32, tag="ps", name=f"ps{ch}_{m}_{n}"
                            )
                            for n in range(2)
                        ]
                        for m in range(MCH)
                    ]
                    for k in range(KT):
                        for n in range(2):
                            for m in range(MCH):
                                nc.tensor.matmul(
                                    psc[m][n][:],
                                    lhsT=xt[k][:, m * 128 : (m + 1) * 128],
                                    rhs=wt[k][n][:],
                                    start=(k == 0),
                                    stop=(k == KT - 1),
                                )
                    for m in range(MCH):
                        for n in range(2):
                            store(psc[m][n], (ch * MCH + m) * 128, n)
                else:
                    # k-inner (psum-sequential): staggers PSUM-bank reuse so
                    # copies overlap the next psum's matmuls
                    for m in range(MCH):
                        row = (ch * MCH + m) * 128
                        for n in range(2):
                            ps = pp.tile([128, 512], f32, tag="ps")
                            for k in range(KT):
                                nc.tensor.matmul(
                                    ps[:],
                                    lhsT=xt[k][:, m * 128 : (m + 1) * 128],
                                    rhs=wt[k][n][:],
                                    start=(k == 0),
                                    stop=(k == KT - 1),
                                )
                            store(ps, row, n)

            emit_chunk(0, xt0, k_outer=True)
            for ch in range(1, NCH):
                xt = []
                for k in range(KT):
                    xk = xp.tile([128, MCH * 128], dt_in, tag="x")
                    nc.sync.dma_start(
                        out=xk[:],
                        in_=xT[k, :, ch * MCH * 128 : (ch + 1) * MCH * 128],
                    )
                    xt.append(xk)
                emit_chunk(ch, xt, k_outer=(ch == 1))
    nc.compile()
    return nc


def _build_dense():
    """Fallback: y = x @ W^T with dense W [4096, 4096] in fp32; any layout.

    Inputs:  xT [32, 128, 1024]   (x transposed, 1024 tokens/core)
             wT [32, 128, 4096]   (W^T = [in, out])
    Output:  y  [1024, 4096]
    """
    nc = bacc.Bacc()
    f32 = mybir.dt.float32
    KT, NO, TOK = 32, 4096, N_TOK // N_CORES
    xT = nc.dram_tensor("xT", [KT, 128, TOK], f32, kind="ExternalInput")
    wT = nc.dram_tensor("wT", [KT, 128, NO], f32, kind="ExternalInput")
    y = nc.dram_tensor("y", [TOK, NO], f32, kind="ExternalOutput")
    MT = TOK // 128
    NT = NO // 512

    with TileContext(nc) as tc:
        with (
            tc.tile_pool(name="xp", bufs=2 * KT) as xp,
            tc.tile_pool(name="wp", bufs=2 * KT) as wp,
            tc.tile_pool(name="pp", bufs=8, space="PSUM") as pp,
            tc.tile_pool(name="op", bufs=8) as op,
        ):
            # n-outer: one 512-wide W panel (32 k-tiles = 64KB/partition)
            # resident at a time; x streamed per m-tile (re-read per panel)
            for n in range(NT):
                wt = []
                for k in range(KT):
                    wk = wp.tile([128, 512], f32, tag="w")
                    nc.sync.dma_start(out=wk[:], in_=wT[k, :, n * 512 : (n + 1) * 512])
                    wt.append(wk)
                for m in range(MT):
                    xt = []
                    for k in range(KT):
                        xk = xp.tile([128, 128], f32, tag="x")
                        nc.sync.dma_start(
                            out=xk[:], in_=xT[k, :, m * 128 : (m + 1) * 128]
                        )
                        xt.append(xk)
                    ps = pp.tile([128, 512], f32, tag="ps")
                    for k in range(KT):
                        nc.tensor.matmul(
                            ps[:],
                            lhsT=xt[k][:],
                            rhs=wt[k][:],
                            start=(k == 0),
                            stop=(k == KT - 1),
                        )
                    ob = op.tile([128, 512], f32, tag="ob")
                    nc.vector.tensor_copy(ob[:], ps[:])
                    nc.scalar.dma_start(
                        out=y[m * 128 : (m + 1) * 128, n * 512 : (n + 1) * 512],
                        in_=ob[:],
                    )
    nc.compile()
    return nc


def _detect_groups(ri, ci):
    """Group block-rows that share an identical block-column set.

    Returns (groups, blk_id) with exactly 4 groups of 32 rows x 32 cols,
    or None if the structure doesn't decompose that way.
    """
    ri = np.asarray(ri)
    ci = np.asarray(ci)
    if len(ri) != HO * WO // 4:
        return None
    pairs = set(zip(ri.tolist(), ci.tolist()))
    if len(pairs) != len(ri):
        return None  # duplicate blocks: last-write-wins semantics -> fallback
    blk_id = np.full((HO, WO), -1, dtype=np.int64)
    blk_id[ri, ci] = np.arange(len(ri))
    col_sets = {}
    for g in range(HO):
        cols = np.sort(ci[ri == g])
        col_sets.setdefault(tuple(cols.tolist()), []).append(g)
    groups = []
    for cols, rows in col_sets.items():
        if len(rows) != 32 or len(cols) != 32:
            return None
        groups.append((np.array(rows), np.array(cols)))
    if len(groups) != 4:
        return None
    return groups, blk_id


def kernel(x, w_blocks, ri, ci):
    global LAST_RESULT
    x = np.asarray(x, dtype=np.float32)
    w_blocks = np.asarray(w_blocks, dtype=np.float32)
    ri = np.asarray(ri, dtype=np.int64)
    ci = np.asarray(ci, dtype=np.int64)

    det = _detect_groups(ri, ci)
    core_ids = list(range(N_CORES))

    if det is not None:
        groups, blk_id = det
        KT, Mg = 8, 1024
        Kg = KT * 128
        wT = np.empty((4, KT, 128, Mg), dtype=np.float32)
        perm_out = np.empty((4, Mg), dtype=np.int64)
        perm_in = np.empty((4, Kg), dtype=np.int64)
        for g, (rows, cols) in enumerate(groups):
            idx = blk_id[np.ix_(rows, cols)]  # [32, 32] block ids
            # Wg[p, q, bi, bj] = W[rows[p]*32+bi, cols[q]*32+bj]
            # -> [q*32+bj, p*32+bi] = Wg^T as [in, out]
            wT[g] = w_blocks[idx].transpose(1, 3, 0, 2).reshape(KT, 128, Mg)
            perm_out[g] = (rows[:, None] * BLOCK + np.arange(BLOCK)).ravel()
            perm_in[g] = (cols[:, None] * BLOCK + np.arange(BLOCK)).ravel()
        xTfull = np.ascontiguousarray(x.T)  # [in, tok]
        xg = xTfull[perm_in.reshape(-1)].reshape(4, Kg, N_TOK)
        in_maps = []
        for c in core_ids:
            g, h = c // 2, c % 2
            xc = np.ascontiguousarray(
                xg[g, :, h * TOKH : (h + 1) * TOKH]
            ).reshape(KT, 128, TOKH)
            in_maps.append({"xT": xc, "wT": wT[g]})
        nc = _build_hybrid(_mm_dt())
        res = run_bass_kernel_spmd(nc, in_maps, core_ids, trace=_TRACE)
        LAST_RESULT = res
        y = np.empty((N_TOK, HO * BLOCK), dtype=np.float32)
        for c in core_ids:
            g, h = c // 2, c % 2
            y[h * TOKH : (h + 1) * TOKH][:, perm_out[g]] = res.results[c]["y"]
        return y

    # ---- dense fallback: scatter blocks into dense W (last write wins)
    TOK = N_TOK // N_CORES
    Wb = np.zeros((HO, WO, BLOCK, BLOCK), dtype=np.float32)
    Wb[ri, ci] = w_blocks
    W = Wb.transpose(0, 2, 1, 3).reshape(HO * BLOCK, WO * BLOCK)
    wT = np.ascontiguousarray(W.T).reshape(32, 128, 4096)
    xTfull = np.ascontiguousarray(x.T)
    in_maps = []
    for c in core_ids:
        xc = np.ascontiguousarray(xTfull[:, c * TOK : (c + 1) * TOK]).reshape(
            32, 128, TOK
        )
        in_maps.append({"xT": xc, "wT": wT})
    nc = _build_dense()
    res = run_bass_kernel_spmd(nc, in_maps, core_ids, trace=_TRACE)
    LAST_RESULT = res
    return np.concatenate([r["y"] for r in res.results], axis=0)


# revision 20
# speedup vs baseline: 1.0160x; 1.0047x over previous
"""Block-sparse linear (DSD) y = x @ W^T on 8 Trainium2 NeuronCores.

Math: W is [4096, 4096] built from 4096 nonzero 32x32 blocks at block
coords (ri, ci) on a 128x128 block grid. The reference layout is
(gi + gj) % 4 == 0, so block-rows with equal (gi mod 4) share an identical
set of 32 block-columns: the sparse matmul decomposes into 4 dense
[tokens x 1024] @ [1024 x 1024] matmuls -- exactly the 25%-density FLOPs.

Sharding (8 cores): hybrid -- 4 residue groups x 2 token halves. Core
c = g*2 + h computes y[h-half, outcols(g)] = x[h-half, incols(g)] @ Wg^T.
Each core reads only its group's weights (4MB) + its token half of the
gathered x (16MB) and writes 16MB -- 36MB of HBM traffic under ~109us of
f32r PE work. No collectives.

Numerics: matmul operands go through the PE's float32r path (fp32 bits,
reduced-precision multiply, fp32 accumulate in PSUM). Measured on HW:
rel L2 error ~1.5e-4 for K=1024 accumulation (vs ~2.3e-3 for bf16).

Host work: pack w_blocks into dense [in, out] panels, transpose/gather x,
un-permute output columns. If ri/ci do not match the 4-group structure,
fall back to a dense-W fp32 kernel (correct for any layout).
"""

import sys

import numpy as np

if "/opt/trn_rl_repo" not in sys.path:
    sys.path.insert(0, "/opt/trn_rl_repo")

import concourse.bacc as bacc
import concourse.mybir as mybir
from concourse.bass_utils import run_bass_kernel_spmd
from concourse.tile import TileContext

# bass_utils imports antenv.axon_hooks when tracing is requested (e.g. via
# BASS_TRACE=1). Some images lack that module; provide an inert stub so a
# trace request degrades to "no trace" instead of crashing.
try:
    import antenv.axon_hooks  # noqa: F401
except Exception:  # pragma: no cover
    import types

    try:
        import antenv

        _hooks = types.ModuleType("antenv.axon_hooks")
        _hooks._h = None
        _hooks.set_axon_ntff_profile_hook = lambda h: setattr(_hooks, "_h", h)
        _hooks.get_axon_ntff_profile_hook = lambda: _hooks._h
        sys.modules["antenv.axon_hooks"] = _hooks
        antenv.axon_hooks = _hooks
    except Exception:
        pass

BLOCK = 32
HO = 128  # out_features // BLOCK
WO = 128  # in_features  // BLOCK
N_TOK = 8192
N_CORES = 8
TOKH = N_TOK // 2  # tokens per core (token half)

# matmul operand dtype: "f32r" = fast fp32 path (1 cycle/row, ~1.5e-4 rel
# err), "f32" = exact fp32 (4 cycles/row)
MM_DTYPE = "f32r"

# set by test.py to capture a profile; harness never touches these
_TRACE = False
LAST_RESULT = None


def _mm_dt():
    return {
        "f32": mybir.dt.float32,
        "f32r": mybir.dt.float32r,
    }[MM_DTYPE]


def _build_hybrid(dt_in):
    """One residue group per core: y[TOKH, 1024] = xg[TOKH, 1024] @ Wg^T.

    Inputs:  xT [8, 128, TOKH]  (gathered x^T: contraction on partitions)
             wT [8, 128, 1024]  (Wg^T panels: [in, out])
    Output:  y  [TOKH, 1024] fp32, out columns in group-local order.
    """
    nc = bacc.Bacc()
    f32 = mybir.dt.float32
    KT, Mg = 8, 1024
    MCH = 4  # m-tiles (128 tokens) per x chunk
    NCH = TOKH // (MCH * 128)  # chunks
    xT = nc.dram_tensor("xT", [KT, 128, TOKH], dt_in, kind="ExternalInput")
    wT = nc.dram_tensor("wT", [KT, 128, Mg], dt_in, kind="ExternalInput")
    y = nc.dram_tensor("y", [TOKH, Mg], f32, kind="ExternalOutput")

    def store(ps, row, n):
        # copy on DVE, store-issue on ScalarE: keeps the Sync engine's
        # instruction stream free for x-chunk load issues
        ob = op.tile([128, 512], f32, tag="ob")
        nc.vector.tensor_copy(ob[:], ps[:])
        nc.scalar.dma_start(
            out=y[row : row + 128, n * 512 : (n + 1) * 512], in_=ob[:]
        )

    with TileContext(nc) as tc:
        with (
            tc.tile_pool(name="wp", bufs=KT) as wp,
            tc.tile_pool(name="xp", bufs=5 * KT) as xp,
            tc.tile_pool(name="pp", bufs=8, space="PSUM") as pp,
            tc.tile_pool(name="op", bufs=8) as op,
        ):
            # PE warm-up: the HAM clock gate holds the PE at 1.2 GHz until
            # it has been busy ~3.4us. Spin junk matmuls on a memset tile
            # while the first DMAs stream so the real stream starts at
            # 2.4 GHz.
            wu = xp.tile([128, 128], f32, tag="warm", bufs=1)
            nc.gpsimd.memset(wu[:], 0.0)
            wups = pp.tile([128, 128], f32, tag="ps")
            NWU = 4  # each fp32 matmul lowers to a LO/HI pair on HW
            for i in range(NWU):
                nc.tensor.matmul(
                    wups[:],
                    lhsT=wu[:],
                    rhs=wu[:],
                    start=(i == 0),
                    stop=(i == NWU - 1),
                )
            # short bridge chain: keeps the PE busy across the ~3us gap
            # until the first k-row's DMA lands, so the HAM clock gate
            # stays at 2.4 GHz when the real stream starts
            for i in range(3):
                nc.tensor.matmul(
                    wups[:],
                    lhsT=wu[:],
                    rhs=wu[:],
                    start=(i == 0),
                    stop=(i == 2),
                )

            # interleave w / x-chunk0 loads k-by-k so the PE can start on
            # k-row 0 after ~0.75MB instead of waiting for all 6MB
            wt, xt0 = [], []
            for k in range(KT):
                wk = wp.tile([128, Mg], dt_in, tag="w")
                nc.sync.dma_start(out=wk[:], in_=wT[k])
                wt.append(wk)
                xk = xp.tile([128, MCH * 128], dt_in, tag="x")
                nc.sync.dma_start(out=xk[:], in_=xT[k, :, 0 : MCH * 128])
                xt0.append(xk)

            def emit_chunk(ch, xt, k_outer):
                if k_outer:
                    # k-outer over all 8 psum banks: each k-row is unlocked
                    # by (w[k], x[k]) alone, so compute streams with DMA
                    # while the chunk's tiles are still arriving
                    psc = [
                        [
                            pp.tile(
                                [128, 512], f32, tag="ps", name=f"ps{ch}_{m}_{n}"
                            )
                            for n in range(2)
                        ]
                        for m in range(MCH)
                    ]
                    for k in range(KT):
                        for m in range(MCH):
                            for n in range(2):
                                nc.tensor.matmul(
                                    psc[m][n][:],
                                    lhsT=xt[k][:, m * 128 : (m + 1) * 128],
                                    rhs=wt[k][:, n * 512 : (n + 1) * 512],
                                    start=(k == 0),
                                    stop=(k == KT - 1),
                                )
                    for m in range(MCH):
                        for n in range(2):
                            store(psc[m][n], (ch * MCH + m) * 128, n)
                else:
                    # k-inner (psum-sequential): staggers PSUM-bank reuse so
                    # copies overlap the next psum's matmuls
                    for m in range(MCH):
                        row = (ch * MCH + m) * 128
                        for n in range(2):
                            ps = pp.tile([128, 512], f32, tag="ps")
                            for k in range(KT):
                                nc.tensor.matmul(
                                    ps[:],
                                    lhsT=xt[k][:, m * 128 : (m + 1) * 128],
                                    rhs=wt[k][:, n * 512 : (n + 1) * 512],
                                    start=(k == 0),
                                    stop=(k == KT - 1),
                                )
                            store(ps, row, n)

            emit_chunk(0, xt0, k_outer=True)
            for ch in range(1, NCH):
                xt = []
                for k in range(KT):
                    xk = xp.tile([128, MCH * 128], dt_in, tag="x")
                    nc.sync.dma_start(
                        out=xk[:],
                        in_=xT[k, :, ch * MCH * 128 : (ch + 1) * MCH * 128],
                    )
                    xt.append(xk)
                emit_chunk(ch, xt, k_outer=(ch == 1))
    nc.compile()
    return nc


def _build_dense():
    """Fallback: y = x @ W^T with dense W [4096, 4096] in fp32; any layout.

    Inputs:  xT [32, 128, 1024]   (x transposed, 1024 tokens/core)
             wT [32, 128, 4096]   (W^T = [in, out])
    Output:  y  [1024, 4096]
    """
    nc = bacc.Bacc()
    f32 = mybir.dt.float32
    KT, NO, TOK = 32, 4096, N_TOK // N_CORES
    xT = nc.dram_tensor("xT", [KT, 128, TOK], f32, kind="ExternalInput")
    wT = nc.dram_tensor("wT", [KT, 128, NO], f32, kind="ExternalInput")
    y = nc.dram_tensor("y", [TOK, NO], f32, kind="ExternalOutput")
    MT = TOK // 128
    NT = NO // 512

    with TileContext(nc) as tc:
        with (
            tc.tile_pool(name="xp", bufs=2 * KT) as xp,
            tc.tile_pool(name="wp", bufs=KT) as wp,
            tc.tile_pool(name="pp", bufs=8, space="PSUM") as pp,
            tc.tile_pool(name="op", bufs=8) as op,
        ):
            # n-outer: one 512-wide W panel (32 k-tiles = 64KB/partition)
            # resident at a time; x streamed per m-tile (re-read per panel)
            for n in range(NT):
                wt = []
                for k in range(KT):
                    wk = wp.tile([128, 512], f32, tag="w")
                    nc.sync.dma_start(out=wk[:], in_=wT[k, :, n * 512 : (n + 1) * 512])
                    wt.append(wk)
                for m in range(MT):
                    xt = []
                    for k in range(KT):
                        xk = xp.tile([128, 128], f32, tag="x")
                        nc.sync.dma_start(
                            out=xk[:], in_=xT[k, :, m * 128 : (m + 1) * 128]
                        )
                        xt.append(xk)
                    ps = pp.tile([128, 512], f32, tag="ps")
                    for k in range(KT):
                        nc.tensor.matmul(
                            ps[:],
                            lhsT=xt[k][:],
                            rhs=wt[k][:],
                            start=(k == 0),
                            stop=(k == KT - 1),
                        )
                    ob = op.tile([128, 512], f32, tag="ob")
                    nc.vector.tensor_copy(ob[:], ps[:])
                    nc.scalar.dma_start(
                        out=y[m * 128 : (m + 1) * 128, n * 512 : (n + 1) * 512],
                        in_=ob[:],
                    )
    nc.compile()
    return nc


def _detect_groups(ri, ci):
    """Group block-rows that share an identical block-column set.

    Returns (groups, blk_id) with exactly 4 groups of 32 rows x 32 cols,
    or None if the structure doesn't decompose that way.
    """
    ri = np.asarray(ri)
    ci = np.asarray(ci)
    if len(ri) != HO * WO // 4:
        return None
    pairs = set(zip(ri.tolist(), ci.tolist()))
    if len(pairs) != len(ri):
        return None  # duplicate blocks: last-write-wins semantics -> fallback
    blk_id = np.full((HO, WO), -1, dtype=np.int64)
    blk_id[ri, ci] = np.arange(len(ri))
    col_sets = {}
    for g in range(HO):
        cols = np.sort(ci[ri == g])
        col_sets.setdefault(tuple(cols.tolist()), []).append(g)
    groups = []
    for cols, rows in col_sets.items():
        if len(rows) != 32 or len(cols) != 32:
            return None
        groups.append((np.array(rows), np.array(cols)))
    if len(groups) != 4:
        return None
    return groups, blk_id


def kernel(x, w_blocks, ri, ci):
    global LAST_RESULT
    x = np.asarray(x, dtype=np.float32)
    w_blocks = np.asarray(w_blocks, dtype=np.float32)
    ri = np.asarray(ri, dtype=np.int64)
    ci = np.asarray(ci, dtype=np.int64)

    det = _detect_groups(ri, ci)
    core_ids = list(range(N_CORES))

    if det is not None:
        groups, blk_id = det
        KT, Mg = 8, 1024
        Kg = KT * 128
        wT = np.empty((4, KT, 128, Mg), dtype=np.float32)
        perm_out = np.empty((4, Mg), dtype=np.int64)
        perm_in = np.empty((4, Kg), dtype=np.int64)
        for g, (rows, cols) in enumerate(groups):
            idx = blk_id[np.ix_(rows, cols)]  # [32, 32] block ids
            # Wg[p, q, bi, bj] = W[rows[p]*32+bi, cols[q]*32+bj]
            # -> [q*32+bj, p*32+bi] = Wg^T as [in, out]
            wT[g] = w_blocks[idx].transpose(1, 3, 0, 2).reshape(KT, 128, Mg)
            perm_out[g] = (rows[:, None] * BLOCK + np.arange(BLOCK)).ravel()
            perm_in[g] = (cols[:, None] * BLOCK + np.arange(BLOCK)).ravel()
        xTfull = np.ascontiguousarray(x.T)  # [in, tok]
        xg = xTfull[perm_in.reshape(-1)].reshape(4, Kg, N_TOK)
        in_maps = []
        for c in core_ids:
            g, h = c // 2, c % 2
            xc = np.ascontiguousarray(
                xg[g, :, h * TOKH : (h + 1) * TOKH]
            ).reshape(KT, 128, TOKH)
            in_maps.append({"xT": xc, "wT": wT[g]})
        nc = _build_hybrid(_mm_dt())
        res = run_bass_kernel_spmd(nc, in_maps, core_ids, trace=_TRACE)
        LAST_RESULT = res
        y = np.empty((N_TOK, HO * BLOCK), dtype=np.float32)
        for c in core_ids:
            g, h = c // 2, c % 2
            y[h * TOKH : (h + 1) * TOKH][:, perm_out[g]] = res.results[c]["y"]
        return y

    # ---- dense fallback: scatter blocks into dense W (last write wins)
    TOK = N_TOK // N_CORES
    Wb = np.zeros((HO, WO, BLOCK, BLOCK), dtype=np.float32)
    Wb[ri, ci] = w_blocks
    W = Wb.transpose(0, 2, 1, 3).reshape(HO * BLOCK, WO * BLOCK)
    wT = np.ascontiguousarray(W.T).reshape(32, 128, 4096)
    xTfull = np.ascontiguousarray(x.T)
    in_maps = []
    for c in core_ids:
        xc = np.ascontiguousarray(xTfull[:, c * TOK : (c + 1) * TOK]).reshape(
            32, 128, TOK
        )
        in_maps.append({"xT": xc, "wT": wT})
    nc = _build_dense()
    res = run_bass_kernel_spmd(nc, in_maps, core_ids, trace=_TRACE)
    LAST_RESULT = res
    return np.concatenate([r["y"] for r in res.results], axis=0)
